# revision 8
# baseline (speedup 1.0000x reference)
"""Trainium2 Bass kernel for linear attention (elu+1 feature map).

Reference computation (B=4, N=M=8192, C=512, H=8, D=64):
    kv   = ref @ kv_w.T              -> k, v  [B,H,N,D]
    q    = tgt @ q_w.T               -> [B,H,M,D];  q,k -> elu(x)+1
    ctx  = sum_n k v^T per head      -> [B,H,D,D];  ksum = sum_n k
    x    = (q @ ctx) * SCALE / (1e-6 + q . ksum)
    out  = x @ proj_w.T + proj_b     -> [B,M,C]

Sharding: 8 cores = 4 batches x 2 row-halves. Each core computes partial
ctx/ksum from its half of N, pair-AllReduces the tiny per-head state, then
produces its half of M rows of the output.

v2 design notes:
  * The three big projections (kv, q, out) run in fp8e4 DoubleRow mode:
    weights are host-scaled x16 into e4m3 normal range and laid out as
    [P, 2, cols] (two 128-deep contraction subtiles per matmul), halving
    both PE cycles and LDWEIGHTS count. The 1/16 descale is folded into
    the ACT `scale=` of the epilogues / the output bias STT.
  * elu(x)+1 = min(exp(x),1) + relu(x): exp reads PSUM directly on ACT
    (no overflow: |x| <~ 3), relu runs in parallel on Pool/DVE, and the
    combine is a cheap all-SBUF STT on DVE.
  * reciprocal via the single-instruction approx-fast DVE op (~51 ULP)
    instead of the ~6 cycles/elem iterative RECIPROCAL.
  * epilogue work is split across ACT/DVE/Pool by parity knobs to keep
    all three engines ~equally loaded; output is bf16 (halves out DMA).
"""

import numpy as np
import ml_dtypes

import concourse.bass as bass
import concourse.mybir as mybir
from concourse import bacc
from concourse.tile import TileContext
from concourse.bass import ts
from concourse.bass_utils import run_bass_kernel_spmd

B, N, M, C, H = 4, 8192, 8192, 512, 8
D = C // H
SCALE = D**-0.5
NCORES = 8
BF = mybir.dt.bfloat16
F32 = mybir.dt.float32
F8 = mybir.dt.float8e4
WS = 16.0          # host weight scale (power of 2; exact)
IWS = 1.0 / WS
XS = 64.0          # xt pre-quantization scale (keeps x out of e4m3 subnormals)

_CACHE = {}


def build(R_ref, R_q, num_devices, replica_groups, lookahead=3):
    """Emit the SPMD graph. R_ref/R_q = rows of the ref/target shard."""
    P = 128
    KC = C // P          # 4 c-chunks (bf16-side tiling: Ksel, ctx pairs)
    KC2 = 2              # fp8 DoubleRow c-subtile pairs (512 = 2 * 2*128)
    NT1 = R_ref // P     # phase-1 row tiles
    CH = 512             # phase-2 chunk (columns of rows)
    NCH = R_q // CH      # phase-2 chunks
    RT = CH // P         # row tiles per chunk
    NPAIR = H // 2       # head pairs
    CP = C + NPAIR       # 516: 4 pairs x 129 cols (128 ctx + 1 ksum)
    STATE = P * CP       # collective payload floats
    DR = mybir.MatmulPerfMode.DoubleRow
    AF = mybir.ActivationFunctionType
    OP = mybir.AluOpType

    nc = bacc.Bacc("TRN2", target_bir_lowering=False, debug=False,
                   num_devices=num_devices)

    refT8 = nc.dram_tensor("refT8", [KC2, P, 2, R_ref], F8, kind="ExternalInput")
    tgtT8 = nc.dram_tensor("tgtT8", [KC2, P, 2, R_q], F8, kind="ExternalInput")
    kvw8 = nc.dram_tensor("kvw8", [KC2, P, 2, 2 * C], F8, kind="ExternalInput")
    qw8 = nc.dram_tensor("qw8", [KC2, P, 2, C], F8, kind="ExternalInput")
    pw8 = nc.dram_tensor("pw8", [KC2, P, 2, C], F8, kind="ExternalInput")
    bias_b = nc.dram_tensor("bias_b", [P, C], BF, kind="ExternalInput")
    out_ext = nc.dram_tensor("out", [R_q, C], BF, kind="ExternalOutput")
    cc_in = nc.dram_tensor("cc_in", [STATE], F32)
    cc_out = nc.dram_tensor("cc_out", [STATE], F32)

    with TileContext(nc) as tc:
        with (
            tc.tile_pool(name="res", bufs=1) as res,
            tc.tile_pool(name="mm", bufs=3, space="PSUM") as pmm,
            tc.tile_pool(name="kv", bufs=4) as kvp,
            tc.tile_pool(name="tmp", bufs=6) as tmp,
            tc.tile_pool(name="rc", bufs=3) as rcp,
            tc.tile_pool(name="qte", bufs=1) as qtep,
            tc.tile_pool(name="xt", bufs=2 * (1 + lookahead)) as xtp,
            tc.tile_pool(name="o", bufs=6) as op_,
        ):
            # ---- resident inputs ----
            # kv weights + refT pieces first so phase 1 can start early.
            NPIECE = 4
            PC_R = R_ref // NPIECE
            PC_Q = R_q // NPIECE
            kvw_sb = []
            for k2 in range(KC2):
                t = res.tile([P, 2, 2 * C], F8, tag=f"kvw{k2}")
                nc.sync.dma_start(t[:], kvw8[k2])
                kvw_sb.append(t)
            refT_sb = [res.tile([P, 2, R_ref], F8, tag=f"refT{k2}",
                                name=f"refT_sb{k2}") for k2 in range(KC2)]
            for pc in range(NPIECE):
                for k2 in range(KC2):
                    nc.sync.dma_start(refT_sb[k2][:, :, ts(pc, PC_R)],
                                      refT8[k2][:, :, ts(pc, PC_R)])
            qw_sb = []
            pw_sb = []
            for k2 in range(KC2):
                t = res.tile([P, 2, C], F8, tag=f"qw{k2}")
                nc.sync.dma_start(t[:], qw8[k2])
                qw_sb.append(t)
                t = res.tile([P, 2, C], F8, tag=f"pw{k2}")
                nc.sync.dma_start(t[:], pw8[k2])
                pw_sb.append(t)
            tgtT_sb = [res.tile([P, 2, R_q], F8, tag=f"tgtT{k2}",
                                name=f"tgtT_sb{k2}") for k2 in range(KC2)]
            for pc in range(NPIECE):
                for k2 in range(KC2):
                    nc.sync.dma_start(tgtT_sb[k2][:, :, ts(pc, PC_Q)],
                                      tgtT8[k2][:, :, ts(pc, PC_Q)])
            bias_sb = res.tile([P, C], BF, tag="bias")
            nc.sync.dma_start(bias_sb[:], bias_b[:, :])
            # zero-init of cc-dependent tiles hoisted here
            ctxs_bd = res.tile([P, C], BF, tag="ctxs_bd")
            nc.vector.memset(ctxs_bd[:], 0.0)
            Ksel = []
            for kc in range(KC):
                s = res.tile([P, H], BF, tag=f"Ksel{kc}", name=f"Ksel{kc}")
                nc.vector.memset(s[:], 0.0)
                Ksel.append(s)

            # ---- phase 1: kv, elu(k), ctx+ksum ----
            VN = 3
            v_res = [res.tile([P, CP], BF, tag=f"vres{r}", name=f"v_res{r}")
                     for r in range(VN)]
            for r in range(VN):
                ones_view = v_res[r][:].rearrange(
                    "p (g c) -> p g c", c=P + 1)[:, :, P : P + 1]
                nc.vector.memset(ones_view, 1.0)

            qte = [[None] * KC for _ in range(NCH)]

            def qt_chunk(j):
                for mc in range(KC):
                    pq = pmm.tile([P, CH], F32, tag="mm")
                    for k2 in range(KC2):
                        nc.tensor.matmul(pq[:], qw_sb[k2][:, :, ts(mc, P)],
                                         tgtT_sb[k2][:, :, ts(j, CH)],
                                         start=(k2 == 0), stop=(k2 == KC2 - 1),
                                         perf_mode=DR)
                    ex = tmp.tile([P, CH], BF, tag="ex")
                    nc.scalar.activation(ex[:], pq[:], AF.Exp, scale=IWS)
                    rq = tmp.tile([P, CH], BF, tag="rq")
                    nc.scalar.activation(rq[:], pq[:], AF.Relu, scale=IWS)
                    exm = tmp.tile([P, CH], BF, tag="exm")
                    nc.vector.tensor_scalar(exm[:], ex[:], 1.0, None, OP.min)
                    q_sb = qtep.tile([P, CH], BF, tag=f"qte{j}_{mc}",
                                     name=f"qte{j}_{mc}")
                    nc.gpsimd.tensor_tensor(q_sb[:], exm[:], rq[:], OP.add)
                    qte[j][mc] = q_sb

            pacc = tc.alloc_tile_pool(name="acc", bufs=1, space="PSUM")
            ctx_ps = [pacc.tile([P, P + 1], F32, tag=f"ctx{p}",
                                name=f"ctx_ps{p}") for p in range(NPAIR)]
            for i in range(NT1):
                pk = pmm.tile([P, C], F32, tag="mm")
                pv = pmm.tile([P, C], F32, tag="mm")
                for k2 in range(KC2):
                    lhsT = refT_sb[k2][:, :, ts(i, P)]
                    nc.tensor.matmul(pk[:], lhsT, kvw_sb[k2][:, :, 0:C],
                                     start=(k2 == 0), stop=(k2 == KC2 - 1),
                                     perf_mode=DR)
                    nc.tensor.matmul(pv[:], lhsT, kvw_sb[k2][:, :, C : 2 * C],
                                     start=(k2 == 0), stop=(k2 == KC2 - 1),
                                     perf_mode=DR)
                # elu(x)+1 = min(exp(x),1) + relu(x); exp safe: |x| <~ 3
                ex = tmp.tile([P, C], BF, tag="ex")
                nc.scalar.activation(ex[:], pk[:], AF.Exp, scale=IWS)
                rk = tmp.tile([P, C], BF, tag="rk")
                nc.vector.tensor_scalar(rk[:], pk[:], IWS, 0.0,
                                        OP.mult, OP.max)
                exm = tmp.tile([P, C], BF, tag="exm")
                nc.vector.tensor_scalar(exm[:], ex[:], 1.0, None, OP.min)
                k_sb = kvp.tile([P, C], BF, tag="k")
                nc.gpsimd.tensor_tensor(k_sb[:], exm[:], rk[:], OP.add)
                v_sb = v_res[i % VN]
                v_view = v_sb[:].rearrange("p (g c) -> p g c",
                                           c=P + 1)[:, :, 0:P]
                nc.scalar.activation(
                    v_view, pv[:].rearrange("p (g c) -> p g c", c=P),
                    AF.Copy, scale=IWS)
                for p in range(NPAIR):
                    nc.tensor.matmul(
                        ctx_ps[p][:], k_sb[:, ts(p, P)],
                        v_sb[:, p * (P + 1) : (p + 1) * (P + 1)],
                        start=(i == 0), stop=(i == NT1 - 1))

            # ---- collective: pair AllReduce of ctx + ksum ----
            ctx_cp = res.tile([P, CP], F32, tag="ctx_cp")
            for p in range(NPAIR):
                nc.scalar.activation(ctx_cp[:, ts(p, P + 1)], ctx_ps[p][:],
                                     AF.Copy)
            pacc.release()
            nc.sync.dma_start(
                cc_in[:].rearrange("(p f) -> p f", p=P), ctx_cp[:])
            nc.gpsimd.collective_compute(
                "AllReduce", mybir.AluOpType.add,
                replica_groups=replica_groups,
                ins=[cc_in[:]], outs=[cc_out[:]])

            def build_state():
                ctxr = res.tile([P, CP], F32, tag="ctxr", name="ctxr")
                nc.sync.dma_start(
                    ctxr[:], cc_out[:].rearrange("(p f) -> p f", p=P))
                for p in range(NPAIR):
                    q0 = p * (P + 1)
                    nc.gpsimd.tensor_copy(ctxs_bd[0:D, p * P : p * P + D],
                                          ctxr[0:D, q0 : q0 + D])
                    nc.gpsimd.tensor_copy(
                        ctxs_bd[D:P, p * P + D : (p + 1) * P],
                        ctxr[D:P, q0 + D : q0 + P])
                for kc in range(KC):
                    kq = kc * (P + 1) + P
                    nc.gpsimd.tensor_copy(Ksel[kc][0:D, 2 * kc : 2 * kc + 1],
                                          ctxr[0:D, kq : kq + 1])
                    nc.gpsimd.tensor_copy(
                        Ksel[kc][D:P, 2 * kc + 1 : 2 * kc + 2],
                        ctxr[D:P, kq + 1 - 1 : kq + 1])

            # ---- phase 2b ----
            paux = tc.alloc_tile_pool(name="aux", bufs=1, space="PSUM")

            recb_all = res.tile([H, R_q], BF, tag="recb_all")
            rb_pair = [res.tile([P, R_q], BF, tag=f"rbp{p}",
                                name=f"rb_pair{p}") for p in range(NPAIR)]

            def den_pass():
                # all chunks' denominators -> recb_all, then 8 bulk
                # broadcast DMAs expand head rows to partition blocks.
                for j in range(NCH):
                    den = paux.tile([H, CH], F32, tag="den", bufs=2,
                                    name="den")
                    for kc in range(KC):
                        nc.tensor.matmul(den[:], Ksel[kc][:], qte[j][kc][:],
                                         start=(kc == 0), stop=(kc == KC - 1))
                    rec = rcp.tile([H, CH], F32, tag="rec")
                    nc.vector.reciprocal_approx_fast(rec[:], den[:])
                    nc.vector.tensor_scalar_mul(recb_all[:, ts(j, CH)],
                                                rec[:], SCALE * XS)
                for p in range(NPAIR):
                    nc.sync.dma_start(
                        rb_pair[p][0:D, :],
                        recb_all[2 * p : 2 * p + 1, :].unsqueeze(1)
                        .broadcast_to([1, D, R_q]))
                    nc.sync.dma_start(
                        rb_pair[p][D:P, :],
                        recb_all[2 * p + 1 : 2 * p + 2, :].unsqueeze(1)
                        .broadcast_to([1, P - D, R_q]))

            def stage_a(j):
                pxs = []
                for p in range(NPAIR):
                    px = paux.tile([P, CH], F32, tag="px", bufs=3,
                                   name="px")
                    nc.tensor.matmul(px[:], ctxs_bd[:, ts(p, P)],
                                     qte[j][p][:], start=True, stop=True)
                    pxs.append(px)
                xt2 = [xtp.tile([P, 2, CH], F8, tag=f"xt{g}",
                                name=f"xt2_{g}")
                       for g in range(2)]
                for p in range(NPAIR):
                    nc.vector.tensor_mul(xt2[p // 2][:, p % 2, :],
                                         pxs[p][:], rb_pair[p][:, ts(j, CH)])
                return xt2

            def stage_b(j, xt2):
                o_sb = op_.tile([P, RT, C], BF, tag="o", bufs=3)
                for rt in range(RT):
                    po = pmm.tile([P, C], F32, tag="mm", name="po")
                    for g in range(2):
                        nc.tensor.matmul(po[:], xt2[g][:, :, ts(rt, P)],
                                         pw_sb[g][:], start=(g == 0),
                                         stop=(g == 1), perf_mode=DR)
                    nc.vector.scalar_tensor_tensor(
                        o_sb[:, rt, :], po[:], 1.0 / (WS * XS), bias_sb[:],
                        OP.mult, OP.add)
                nc.sync.dma_start(
                    out_ext[ts(j, CH), :].rearrange("(rt p) c -> p rt c",
                                                    p=P),
                    o_sb[:])

            for j in range(NCH):
                if j == max(NCH - 2, 0):
                    build_state()
                qt_chunk(j)
            den_pass()
            pend = []
            for j in range(NCH):
                pend.append((j, stage_a(j)))
                if len(pend) > lookahead:
                    jj, xx = pend.pop(0)
                    stage_b(jj, xx)
            for jj, xx in pend:
                stage_b(jj, xx)
            paux.release()
    nc.compile()
    return nc


def _sub2(a):
    """[C, X] -> [KC2, 128, 2, X] DoubleRow c-subtile layout."""
    X = a.shape[1]
    return np.ascontiguousarray(
        a.reshape(2, 2, 128, X).transpose(0, 2, 1, 3))


def _shard_inputs(target_data, reference_data, q_w, kv_w, proj_w, proj_b,
                  R, ncores):
    bf = ml_dtypes.bfloat16
    f8 = ml_dtypes.float8_e4m3
    kv_wT = np.asarray(kv_w).T.astype(np.float32) * WS
    q_wT = np.asarray(q_w).T.astype(np.float32) * WS
    proj_wT = np.asarray(proj_w).T.astype(np.float32) * WS
    kvw8 = _sub2(kv_wT).astype(f8)
    qw8 = _sub2(q_wT).astype(f8)
    pw8 = _sub2(proj_wT).astype(f8)
    bias_b = np.ascontiguousarray(
        np.broadcast_to(np.asarray(proj_b)[None, :], (128, C))).astype(bf)
    in_maps = []
    for c in range(ncores):
        b, half = divmod(c, 2)
        sl = slice(half * R, (half + 1) * R)
        in_maps.append({
            "refT8": _sub2(np.ascontiguousarray(
                np.asarray(reference_data)[b, sl, :].T)).astype(f8),
            "tgtT8": _sub2(np.ascontiguousarray(
                np.asarray(target_data)[b, sl, :].T)).astype(f8),
            "kvw8": kvw8, "qw8": qw8, "pw8": pw8,
            "bias_b": bias_b,
        })
    return in_maps


def kernel(target_data, reference_data, q_w, kv_w, proj_w, proj_b):
    R = M // 2
    key = (R, NCORES)
    if key not in _CACHE:
        _CACHE[key] = build(R, R, NCORES,
                            [[0, 1], [2, 3], [4, 5], [6, 7]], lookahead=3)
    nc = _CACHE[key]
    in_maps = _shard_inputs(target_data, reference_data, q_w, kv_w, proj_w,
                            proj_b, R, NCORES)
    res = run_bass_kernel_spmd(nc, in_maps, list(range(NCORES)))
    out = np.empty((B, M, C), dtype=np.float32)
    for c in range(NCORES):
        b, half = divmod(c, 2)
        out[b, half * R : (half + 1) * R, :] = np.asarray(
            res.results[c]["out"]).astype(np.float32)
    return out


# revision 9
# speedup vs baseline: 1.6406x; 1.6406x over previous
"""Trainium2 Bass kernel for linear attention (elu+1 feature map).

Reference computation (B=4, N=M=8192, C=512, H=8, D=64):
    kv   = ref @ kv_w.T              -> k, v  [B,H,N,D]
    q    = tgt @ q_w.T               -> [B,H,M,D];  q,k -> elu(x)+1
    ctx  = sum_n k v^T per head      -> [B,H,D,D];  ksum = sum_n k
    x    = (q @ ctx) * SCALE / (1e-6 + q . ksum)
    out  = x @ proj_w.T + proj_b     -> [B,M,C]

Sharding: 8 cores = 4 batches x 2 row-halves. Each core computes partial
ctx/ksum from its half of N, pair-AllReduces the tiny per-head state, then
produces its half of M rows of the output.

v2 design notes:
  * The three big projections (kv, q, out) run in fp8e4 DoubleRow mode:
    weights are host-scaled x16 into e4m3 normal range and laid out as
    [P, 2, cols] (two 128-deep contraction subtiles per matmul), halving
    both PE cycles and LDWEIGHTS count. The 1/16 descale is folded into
    the ACT `scale=` of the epilogues / the output bias STT.
  * elu(x)+1 = min(exp(x),1) + relu(x): exp reads PSUM directly on ACT
    (no overflow: |x| <~ 3), relu runs in parallel on Pool/DVE, and the
    combine is a cheap all-SBUF STT on DVE.
  * reciprocal via the single-instruction approx-fast DVE op (~51 ULP)
    instead of the ~6 cycles/elem iterative RECIPROCAL.
  * epilogue work is split across ACT/DVE/Pool by parity knobs to keep
    all three engines ~equally loaded; output is bf16 (halves out DMA).
"""

import numpy as np
import ml_dtypes

import concourse.bass as bass
import concourse.mybir as mybir
from concourse import bacc
from concourse.tile import TileContext
from concourse.bass import ts
from concourse.bass_utils import run_bass_kernel_spmd

B, N, M, C, H = 4, 8192, 8192, 512, 8
D = C // H
SCALE = D**-0.5
NCORES = 8
BF = mybir.dt.bfloat16
F32 = mybir.dt.float32
F8 = mybir.dt.float8e4
WS = 16.0          # host weight scale (power of 2; exact)
IWS = 1.0 / WS
XS = 64.0          # xt pre-quantization scale (keeps x out of e4m3 subnormals)

_CACHE = {}


def build(R_ref, R_q, num_devices, replica_groups, lookahead=3):
    """Emit the SPMD graph. R_ref/R_q = rows of the ref/target shard."""
    P = 128
    KC = C // P          # 4 c-chunks (bf16-side tiling: Ksel, ctx pairs)
    KC2 = 2              # fp8 DoubleRow c-subtile pairs (512 = 2 * 2*128)
    NT1 = R_ref // P     # phase-1 row tiles
    CH = 512             # phase-2 chunk (columns of rows)
    NCH = R_q // CH      # phase-2 chunks
    RT = CH // P         # row tiles per chunk
    NPAIR = H // 2       # head pairs
    CP = C + NPAIR       # 516: 4 pairs x 129 cols (128 ctx + 1 ksum)
    STATE = P * CP       # collective payload floats
    DR = mybir.MatmulPerfMode.DoubleRow
    AF = mybir.ActivationFunctionType
    OP = mybir.AluOpType

    nc = bacc.Bacc("TRN2", target_bir_lowering=False, debug=False,
                   num_devices=num_devices)

    refT8 = nc.dram_tensor("refT8", [KC2, P, 2, R_ref], F8, kind="ExternalInput")
    tgtT8 = nc.dram_tensor("tgtT8", [KC2, P, 2, R_q], F8, kind="ExternalInput")
    kvw8 = nc.dram_tensor("kvw8", [KC2, P, 2, 2 * C], F8, kind="ExternalInput")
    qw8 = nc.dram_tensor("qw8", [KC2, P, 2, C], F8, kind="ExternalInput")
    pw8 = nc.dram_tensor("pw8", [KC2, P, 2, C], F8, kind="ExternalInput")
    bias_b = nc.dram_tensor("bias_b", [P, C], BF, kind="ExternalInput")
    E_const = nc.dram_tensor("E_const", [NPAIR, H, P], BF, kind="ExternalInput")
    out_ext = nc.dram_tensor("out", [R_q, C], BF, kind="ExternalOutput")
    cc_in = nc.dram_tensor("cc_in", [STATE], F32)
    cc_out = nc.dram_tensor("cc_out", [STATE], F32)

    with TileContext(nc) as tc:
        with (
            tc.tile_pool(name="res", bufs=1) as res,
            tc.tile_pool(name="mm", bufs=3, space="PSUM") as pmm,
            tc.tile_pool(name="kv", bufs=4) as kvp,
            tc.tile_pool(name="tmp", bufs=6) as tmp,
            tc.tile_pool(name="rc", bufs=3) as rcp,
            tc.tile_pool(name="qte", bufs=1) as qtep,
            tc.tile_pool(name="xt", bufs=2 * (1 + lookahead)) as xtp,
            tc.tile_pool(name="o", bufs=6) as op_,
        ):
            # ---- resident inputs ----
            # kv weights + refT pieces first so phase 1 can start early.
            NPIECE = 4
            PC_R = R_ref // NPIECE
            PC_Q = R_q // NPIECE
            kvw_sb = []
            for k2 in range(KC2):
                t = res.tile([P, 2, 2 * C], F8, tag=f"kvw{k2}")
                nc.sync.dma_start(t[:], kvw8[k2])
                kvw_sb.append(t)
            refT_sb = [res.tile([P, 2, R_ref], F8, tag=f"refT{k2}",
                                name=f"refT_sb{k2}") for k2 in range(KC2)]
            for pc in range(NPIECE):
                for k2 in range(KC2):
                    nc.sync.dma_start(refT_sb[k2][:, :, ts(pc, PC_R)],
                                      refT8[k2][:, :, ts(pc, PC_R)])
            qw_sb = []
            pw_sb = []
            for k2 in range(KC2):
                t = res.tile([P, 2, C], F8, tag=f"qw{k2}")
                nc.sync.dma_start(t[:], qw8[k2])
                qw_sb.append(t)
                t = res.tile([P, 2, C], F8, tag=f"pw{k2}")
                nc.sync.dma_start(t[:], pw8[k2])
                pw_sb.append(t)
            tgtT_sb = [res.tile([P, 2, R_q], F8, tag=f"tgtT{k2}",
                                name=f"tgtT_sb{k2}") for k2 in range(KC2)]
            for pc in range(NPIECE):
                for k2 in range(KC2):
                    nc.sync.dma_start(tgtT_sb[k2][:, :, ts(pc, PC_Q)],
                                      tgtT8[k2][:, :, ts(pc, PC_Q)])
            bias_sb = res.tile([P, C], BF, tag="bias")
            nc.sync.dma_start(bias_sb[:], bias_b[:, :])
            E_sb = []
            for p in range(NPAIR):
                e = res.tile([H, P], BF, tag=f"E{p}", name=f"E_sb{p}")
                nc.sync.dma_start(e[:], E_const[p])
                E_sb.append(e)
            # zero-init of cc-dependent tiles hoisted here
            ctxs_bd = res.tile([P, C], BF, tag="ctxs_bd")
            nc.vector.memset(ctxs_bd[:], 0.0)
            Ksel = []
            for kc in range(KC):
                s = res.tile([P, H], BF, tag=f"Ksel{kc}", name=f"Ksel{kc}")
                nc.vector.memset(s[:], 0.0)
                Ksel.append(s)

            # ---- phase 1: kv, elu(k), ctx+ksum ----
            VN = 3
            v_res = [res.tile([P, CP], BF, tag=f"vres{r}", name=f"v_res{r}")
                     for r in range(VN)]
            for r in range(VN):
                ones_view = v_res[r][:].rearrange(
                    "p (g c) -> p g c", c=P + 1)[:, :, P : P + 1]
                nc.vector.memset(ones_view, 1.0)

            qte = [[None] * KC for _ in range(NCH)]

            def qt_chunk(j):
                for mc in range(KC):
                    pq = pmm.tile([P, CH], F32, tag="mm")
                    for k2 in range(KC2):
                        nc.tensor.matmul(pq[:], qw_sb[k2][:, :, ts(mc, P)],
                                         tgtT_sb[k2][:, :, ts(j, CH)],
                                         start=(k2 == 0), stop=(k2 == KC2 - 1),
                                         perf_mode=DR)
                    ex = tmp.tile([P, CH], BF, tag="ex")
                    nc.scalar.activation(ex[:], pq[:], AF.Exp, scale=IWS)
                    rq = tmp.tile([P, CH], BF, tag="rq")
                    if mc % 2 == 0:
                        nc.scalar.activation(rq[:], pq[:], AF.Relu, scale=IWS)
                    else:
                        nc.vector.tensor_scalar(rq[:], pq[:], IWS, 0.0,
                                                OP.mult, OP.max)
                    exm = tmp.tile([P, CH], BF, tag="exm")
                    nc.vector.tensor_scalar(exm[:], ex[:], 1.0, None, OP.min)
                    q_sb = qtep.tile([P, CH], BF, tag=f"qte{j}_{mc}",
                                     name=f"qte{j}_{mc}")
                    nc.gpsimd.tensor_tensor(q_sb[:], exm[:], rq[:], OP.add)
                    qte[j][mc] = q_sb

            pacc = tc.alloc_tile_pool(name="acc", bufs=1, space="PSUM")
            ctx_ps = [pacc.tile([P, P + 1], F32, tag=f"ctx{p}",
                                name=f"ctx_ps{p}") for p in range(NPAIR)]
            for i in range(NT1):
                pk = pmm.tile([P, C], F32, tag="mm")
                pv = pmm.tile([P, C], F32, tag="mm")
                for k2 in range(KC2):
                    lhsT = refT_sb[k2][:, :, ts(i, P)]
                    nc.tensor.matmul(pk[:], lhsT, kvw_sb[k2][:, :, 0:C],
                                     start=(k2 == 0), stop=(k2 == KC2 - 1),
                                     perf_mode=DR)
                    nc.tensor.matmul(pv[:], lhsT, kvw_sb[k2][:, :, C : 2 * C],
                                     start=(k2 == 0), stop=(k2 == KC2 - 1),
                                     perf_mode=DR)
                # elu(x)+1 = min(exp(x),1) + relu(x); exp safe: |x| <~ 3
                ex = tmp.tile([P, C], BF, tag="ex")
                nc.scalar.activation(ex[:], pk[:], AF.Exp, scale=IWS)
                rk = tmp.tile([P, C], BF, tag="rk")
                nc.vector.tensor_scalar(rk[:], pk[:], IWS, 0.0,
                                        OP.mult, OP.max)
                exm = tmp.tile([P, C], BF, tag="exm")
                nc.vector.tensor_scalar(exm[:], ex[:], 1.0, None, OP.min)
                k_sb = kvp.tile([P, C], BF, tag="k")
                nc.gpsimd.tensor_tensor(k_sb[:], exm[:], rk[:], OP.add)
                v_sb = v_res[i % VN]
                v_view = v_sb[:].rearrange("p (g c) -> p g c",
                                           c=P + 1)[:, :, 0:P]
                nc.scalar.activation(
                    v_view, pv[:].rearrange("p (g c) -> p g c", c=P),
                    AF.Copy, scale=IWS)
                for p in range(NPAIR):
                    nc.tensor.matmul(
                        ctx_ps[p][:], k_sb[:, ts(p, P)],
                        v_sb[:, p * (P + 1) : (p + 1) * (P + 1)],
                        start=(i == 0), stop=(i == NT1 - 1))

            # ---- collective: pair AllReduce of ctx + ksum ----
            ctx_cp = res.tile([P, CP], F32, tag="ctx_cp")
            for p in range(NPAIR):
                nc.scalar.activation(ctx_cp[:, ts(p, P + 1)], ctx_ps[p][:],
                                     AF.Copy)
            pacc.release()
            nc.sync.dma_start(
                cc_in[:].rearrange("(p f) -> p f", p=P), ctx_cp[:])
            nc.gpsimd.collective_compute(
                "AllReduce", mybir.AluOpType.add,
                replica_groups=replica_groups,
                ins=[cc_in[:]], outs=[cc_out[:]])

            def build_state():
                ctxr = res.tile([P, CP], F32, tag="ctxr", name="ctxr")
                nc.sync.dma_start(
                    ctxr[:], cc_out[:].rearrange("(p f) -> p f", p=P))
                for p in range(NPAIR):
                    q0 = p * (P + 1)
                    nc.gpsimd.tensor_copy(ctxs_bd[0:D, p * P : p * P + D],
                                          ctxr[0:D, q0 : q0 + D])
                    nc.gpsimd.tensor_copy(
                        ctxs_bd[D:P, p * P + D : (p + 1) * P],
                        ctxr[D:P, q0 + D : q0 + P])
                for kc in range(KC):
                    kq = kc * (P + 1) + P
                    nc.gpsimd.tensor_copy(Ksel[kc][0:D, 2 * kc : 2 * kc + 1],
                                          ctxr[0:D, kq : kq + 1])
                    nc.gpsimd.tensor_copy(
                        Ksel[kc][D:P, 2 * kc + 1 : 2 * kc + 2],
                        ctxr[D:P, kq + 1 - 1 : kq + 1])

            # ---- phase 2b ----
            paux = tc.alloc_tile_pool(name="aux", bufs=1, space="PSUM")

            def stage_a(j):
                den = paux.tile([H, CH], F32, tag="rb", bufs=2, name="den")
                for kc in range(KC):
                    nc.tensor.matmul(den[:], Ksel[kc][:], qte[j][kc][:],
                                     start=(kc == 0), stop=(kc == KC - 1))
                rec = rcp.tile([H, CH], F32, tag="rec")
                nc.vector.reciprocal_approx_fast(rec[:], den[:])
                recb = rcp.tile([H, CH], BF, tag="recb")
                nc.vector.tensor_scalar_mul(recb[:], rec[:], SCALE * XS)
                pxs = []
                for p in range(NPAIR):
                    px = paux.tile([P, CH], F32, tag="px", bufs=3,
                                   name="px")
                    nc.tensor.matmul(px[:], ctxs_bd[:, ts(p, P)],
                                     qte[j][p][:], start=True, stop=True)
                    pxs.append(px)
                xt2 = [xtp.tile([P, 2, CH], F8, tag=f"xt{g}",
                                name=f"xt2_{g}")
                       for g in range(2)]
                for p in range(NPAIR):
                    prb = paux.tile([P, CH], F32, tag="rb", bufs=2,
                                    name="prb")
                    nc.tensor.matmul(prb[:], E_sb[p][:], recb[:],
                                     start=True, stop=True)
                    rb = rcp.tile([P, CH], BF, tag="rbs")
                    nc.scalar.activation(rb[:], prb[:], AF.Copy)
                    nc.vector.tensor_mul(xt2[p // 2][:, p % 2, :],
                                         pxs[p][:], rb[:])
                return xt2

            def stage_b(j, xt2):
                o_sb = op_.tile([P, RT, C], BF, tag="o", bufs=3)
                for rt in range(RT):
                    po = pmm.tile([P, C], F32, tag="mm", name="po")
                    for g in range(2):
                        nc.tensor.matmul(po[:], xt2[g][:, :, ts(rt, P)],
                                         pw_sb[g][:], start=(g == 0),
                                         stop=(g == 1), perf_mode=DR)
                    nc.vector.scalar_tensor_tensor(
                        o_sb[:, rt, :], po[:], 1.0 / (WS * XS), bias_sb[:],
                        OP.mult, OP.add)
                nc.sync.dma_start(
                    out_ext[ts(j, CH), :].rearrange("(rt p) c -> p rt c",
                                                    p=P),
                    o_sb[:])

            for j in range(NCH):
                if j == max(NCH - 2, 0):
                    build_state()
                qt_chunk(j)
            pend = []
            for j in range(NCH):
                pend.append((j, stage_a(j)))
                if len(pend) > lookahead:
                    jj, xx = pend.pop(0)
                    stage_b(jj, xx)
            for jj, xx in pend:
                stage_b(jj, xx)
            paux.release()
    nc.compile()
    return nc


def _sub2(a):
    """[C, X] -> [KC2, 128, 2, X] DoubleRow c-subtile layout."""
    X = a.shape[1]
    return np.ascontiguousarray(
        a.reshape(2, 2, 128, X).transpose(0, 2, 1, 3))


def _shard_inputs(target_data, reference_data, q_w, kv_w, proj_w, proj_b,
                  R, ncores):
    bf = ml_dtypes.bfloat16
    f8 = ml_dtypes.float8_e4m3
    kv_wT = np.asarray(kv_w).T.astype(np.float32) * WS
    q_wT = np.asarray(q_w).T.astype(np.float32) * WS
    proj_wT = np.asarray(proj_w).T.astype(np.float32) * WS
    kvw8 = _sub2(kv_wT).astype(f8)
    qw8 = _sub2(q_wT).astype(f8)
    pw8 = _sub2(proj_wT).astype(f8)
    bias_b = np.ascontiguousarray(
        np.broadcast_to(np.asarray(proj_b)[None, :], (128, C))).astype(bf)
    npair = H // 2
    E_const = np.zeros((npair, H, 128), dtype=bf)
    for p in range(npair):
        E_const[p, 2 * p, 0:D] = 1.0
        E_const[p, 2 * p + 1, D:128] = 1.0
    in_maps = []
    for c in range(ncores):
        b, half = divmod(c, 2)
        sl = slice(half * R, (half + 1) * R)
        in_maps.append({
            "refT8": _sub2(np.ascontiguousarray(
                np.asarray(reference_data)[b, sl, :].T)).astype(f8),
            "tgtT8": _sub2(np.ascontiguousarray(
                np.asarray(target_data)[b, sl, :].T)).astype(f8),
            "kvw8": kvw8, "qw8": qw8, "pw8": pw8,
            "bias_b": bias_b, "E_const": E_const,
        })
    return in_maps


def kernel(target_data, reference_data, q_w, kv_w, proj_w, proj_b):
    R = M // 2
    key = (R, NCORES)
    if key not in _CACHE:
        _CACHE[key] = build(R, R, NCORES,
                            [[0, 1], [2, 3], [4, 5], [6, 7]], lookahead=3)
    nc = _CACHE[key]
    in_maps = _shard_inputs(target_data, reference_data, q_w, kv_w, proj_w,
                            proj_b, R, NCORES)
    res = run_bass_kernel_spmd(nc, in_maps, list(range(NCORES)))
    out = np.empty((B, M, C), dtype=np.float32)
    for c in range(NCORES):
        b, half = divmod(c, 2)
        out[b, half * R : (half + 1) * R, :] = np.asarray(
            res.results[c]["out"]).astype(np.float32)
    return out


# revision 10
# speedup vs baseline: 1.8932x; 1.1540x over previous
"""Trainium2 Bass kernel for linear attention (elu+1 feature map).

Reference computation (B=4, N=M=8192, C=512, H=8, D=64):
    kv   = ref @ kv_w.T              -> k, v  [B,H,N,D]
    q    = tgt @ q_w.T               -> [B,H,M,D];  q,k -> elu(x)+1
    ctx  = sum_n k v^T per head      -> [B,H,D,D];  ksum = sum_n k
    x    = (q @ ctx) * SCALE / (1e-6 + q . ksum)
    out  = x @ proj_w.T + proj_b     -> [B,M,C]

Sharding: 8 cores = 4 batches x 2 row-halves. Each core computes partial
ctx/ksum from its half of N, pair-AllReduces the tiny per-head state, then
produces its half of M rows of the output.

v2 design notes:
  * The three big projections (kv, q, out) run in fp8e4 DoubleRow mode:
    weights are host-scaled x16 into e4m3 normal range and laid out as
    [P, 2, cols] (two 128-deep contraction subtiles per matmul), halving
    both PE cycles and LDWEIGHTS count. The 1/16 descale is folded into
    the ACT `scale=` of the epilogues / the output bias STT.
  * elu(x)+1 = min(exp(x),1) + relu(x): exp reads PSUM directly on ACT
    (no overflow: |x| <~ 3), relu runs in parallel on Pool/DVE, and the
    combine is a cheap all-SBUF STT on DVE.
  * reciprocal via the single-instruction approx-fast DVE op (~51 ULP)
    instead of the ~6 cycles/elem iterative RECIPROCAL.
  * epilogue work is split across ACT/DVE/Pool by parity knobs to keep
    all three engines ~equally loaded; output is bf16 (halves out DMA).
"""

import numpy as np
import ml_dtypes

import concourse.bass as bass
import concourse.mybir as mybir
from concourse import bacc
from concourse.tile import TileContext
from concourse.bass import ts
from concourse.bass_utils import run_bass_kernel_spmd

B, N, M, C, H = 4, 8192, 8192, 512, 8
D = C // H
SCALE = D**-0.5
NCORES = 8
BF = mybir.dt.bfloat16
F32 = mybir.dt.float32
F8 = mybir.dt.float8e4
WS = 16.0          # host weight scale (power of 2; exact)
IWS = 1.0 / WS
XS = 64.0          # xt pre-quantization scale (keeps x out of e4m3 subnormals)

_CACHE = {}


def build(R_ref, R_q, num_devices, replica_groups, lookahead=3):
    """Emit the SPMD graph. R_ref/R_q = rows of the ref/target shard."""
    P = 128
    KC = C // P          # 4 c-chunks (bf16-side tiling: Ksel, ctx pairs)
    KC2 = 2              # fp8 DoubleRow c-subtile pairs (512 = 2 * 2*128)
    NT1 = R_ref // P     # phase-1 row tiles
    CH = 512             # phase-2 chunk (columns of rows)
    NCH = R_q // CH      # phase-2 chunks
    RT = CH // P         # row tiles per chunk
    NPAIR = H // 2       # head pairs
    CP = C + NPAIR       # 516: 4 pairs x 129 cols (128 ctx + 1 ksum)
    STATE = P * CP       # collective payload floats
    DR = mybir.MatmulPerfMode.DoubleRow
    AF = mybir.ActivationFunctionType
    OP = mybir.AluOpType

    nc = bacc.Bacc("TRN2", target_bir_lowering=False, debug=False,
                   num_devices=num_devices)

    refT8 = nc.dram_tensor("refT8", [KC2, P, 2, R_ref], F8, kind="ExternalInput")
    tgtT8 = nc.dram_tensor("tgtT8", [KC2, P, 2, R_q], F8, kind="ExternalInput")
    kvw8 = nc.dram_tensor("kvw8", [KC2, P, 2, 2 * C], F8, kind="ExternalInput")
    qw8 = nc.dram_tensor("qw8", [KC2, P, 2, C], F8, kind="ExternalInput")
    pw8 = nc.dram_tensor("pw8", [KC2, P, 2, C], F8, kind="ExternalInput")
    bias_b = nc.dram_tensor("bias_b", [P, C], BF, kind="ExternalInput")
    E_const = nc.dram_tensor("E_const", [NPAIR, H, P], BF, kind="ExternalInput")
    out_ext = nc.dram_tensor("out", [R_q, C], BF, kind="ExternalOutput")
    cc_in1 = nc.dram_tensor("cc_in1", [STATE], F32)
    cc_out1 = nc.dram_tensor("cc_out1", [STATE], F32)
    cc_in2 = nc.dram_tensor("cc_in2", [STATE], F32)
    cc_out2 = nc.dram_tensor("cc_out2", [STATE], F32)

    with TileContext(nc) as tc:
        with (
            tc.tile_pool(name="res", bufs=1) as res,
            tc.tile_pool(name="mm", bufs=3, space="PSUM") as pmm,
            tc.tile_pool(name="kv", bufs=4) as kvp,
            tc.tile_pool(name="tmp", bufs=6) as tmp,
            tc.tile_pool(name="rc", bufs=3) as rcp,
            tc.tile_pool(name="qte", bufs=1) as qtep,
            tc.tile_pool(name="xt", bufs=2 * (1 + lookahead)) as xtp,
            tc.tile_pool(name="o", bufs=6) as op_,
        ):
            # ---- resident inputs ----
            # kv weights + refT pieces first so phase 1 can start early.
            NPIECE = 4
            PC_R = R_ref // NPIECE
            PC_Q = R_q // NPIECE
            kvw_sb = []
            for k2 in range(KC2):
                t = res.tile([P, 2, 2 * C], F8, tag=f"kvw{k2}")
                nc.sync.dma_start(t[:], kvw8[k2])
                kvw_sb.append(t)
            refT_sb = [res.tile([P, 2, R_ref], F8, tag=f"refT{k2}",
                                name=f"refT_sb{k2}") for k2 in range(KC2)]
            for pc in range(NPIECE):
                for k2 in range(KC2):
                    nc.sync.dma_start(refT_sb[k2][:, :, ts(pc, PC_R)],
                                      refT8[k2][:, :, ts(pc, PC_R)])
            qw_sb = []
            pw_sb = []
            for k2 in range(KC2):
                t = res.tile([P, 2, C], F8, tag=f"qw{k2}")
                nc.sync.dma_start(t[:], qw8[k2])
                qw_sb.append(t)
                t = res.tile([P, 2, C], F8, tag=f"pw{k2}")
                nc.sync.dma_start(t[:], pw8[k2])
                pw_sb.append(t)
            tgtT_sb = [res.tile([P, 2, R_q], F8, tag=f"tgtT{k2}",
                                name=f"tgtT_sb{k2}") for k2 in range(KC2)]
            for pc in range(NPIECE):
                for k2 in range(KC2):
                    nc.sync.dma_start(tgtT_sb[k2][:, :, ts(pc, PC_Q)],
                                      tgtT8[k2][:, :, ts(pc, PC_Q)])
            bias_sb = res.tile([P, C], BF, tag="bias")
            nc.sync.dma_start(bias_sb[:], bias_b[:, :])
            E_sb = []
            for p in range(NPAIR):
                e = res.tile([H, P], BF, tag=f"E{p}", name=f"E_sb{p}")
                nc.sync.dma_start(e[:], E_const[p])
                E_sb.append(e)
            # zero-init of cc-dependent tiles hoisted here
            ctxs_bd = res.tile([P, C], BF, tag="ctxs_bd")
            nc.vector.memset(ctxs_bd[:], 0.0)
            Ksel = []
            for kc in range(KC):
                s = res.tile([P, H], BF, tag=f"Ksel{kc}", name=f"Ksel{kc}")
                nc.vector.memset(s[:], 0.0)
                Ksel.append(s)

            # ---- phase 1: kv, elu(k), ctx+ksum ----
            VN = 3
            v_res = [res.tile([P, CP], BF, tag=f"vres{r}", name=f"v_res{r}")
                     for r in range(VN)]
            for r in range(VN):
                ones_view = v_res[r][:].rearrange(
                    "p (g c) -> p g c", c=P + 1)[:, :, P : P + 1]
                nc.vector.memset(ones_view, 1.0)

            qte = [[None] * KC for _ in range(NCH)]

            def qt_chunk(j):
                for mc in range(KC):
                    pq = pmm.tile([P, CH], F32, tag="mm")
                    for k2 in range(KC2):
                        nc.tensor.matmul(pq[:], qw_sb[k2][:, :, ts(mc, P)],
                                         tgtT_sb[k2][:, :, ts(j, CH)],
                                         start=(k2 == 0), stop=(k2 == KC2 - 1),
                                         perf_mode=DR)
                    ex = tmp.tile([P, CH], BF, tag="ex")
                    nc.scalar.activation(ex[:], pq[:], AF.Exp, scale=IWS)
                    rq = tmp.tile([P, CH], BF, tag="rq")
                    if mc % 2 == 0:
                        nc.scalar.activation(rq[:], pq[:], AF.Relu, scale=IWS)
                    else:
                        nc.vector.tensor_scalar(rq[:], pq[:], IWS, 0.0,
                                                OP.mult, OP.max)
                    exm = tmp.tile([P, CH], BF, tag="exm")
                    nc.vector.tensor_scalar(exm[:], ex[:], 1.0, None, OP.min)
                    q_sb = qtep.tile([P, CH], BF, tag=f"qte{j}_{mc}",
                                     name=f"qte{j}_{mc}")
                    nc.gpsimd.tensor_tensor(q_sb[:], exm[:], rq[:], OP.add)
                    qte[j][mc] = q_sb

            pacc = tc.alloc_tile_pool(name="acc", bufs=1, space="PSUM")
            ctx_ps = [pacc.tile([P, P + 1], F32, tag=f"ctx{p}",
                                name=f"ctx_ps{p}") for p in range(NPAIR)]
            HT = NT1 // 2

            def flush_ctx(ctx_cp, cc_in, cc_out):
                for p in range(NPAIR):
                    nc.scalar.activation(ctx_cp[:, ts(p, P + 1)],
                                         ctx_ps[p][:], AF.Copy)
                nc.sync.dma_start(
                    cc_in[:].rearrange("(p f) -> p f", p=P), ctx_cp[:])
                nc.gpsimd.collective_compute(
                    "AllReduce", mybir.AluOpType.add,
                    replica_groups=replica_groups,
                    ins=[cc_in[:]], outs=[cc_out[:]])

            for i in range(NT1):
                pk = pmm.tile([P, C], F32, tag="mm")
                pv = pmm.tile([P, C], F32, tag="mm")
                for k2 in range(KC2):
                    lhsT = refT_sb[k2][:, :, ts(i, P)]
                    nc.tensor.matmul(pk[:], lhsT, kvw_sb[k2][:, :, 0:C],
                                     start=(k2 == 0), stop=(k2 == KC2 - 1),
                                     perf_mode=DR)
                    nc.tensor.matmul(pv[:], lhsT, kvw_sb[k2][:, :, C : 2 * C],
                                     start=(k2 == 0), stop=(k2 == KC2 - 1),
                                     perf_mode=DR)
                # elu(x)+1 = min(exp(x),1) + relu(x); exp safe: |x| <~ 3
                ex = tmp.tile([P, C], BF, tag="ex")
                nc.scalar.activation(ex[:], pk[:], AF.Exp, scale=IWS)
                rk = tmp.tile([P, C], BF, tag="rk")
                nc.vector.tensor_scalar(rk[:], pk[:], IWS, 0.0,
                                        OP.mult, OP.max)
                exm = tmp.tile([P, C], BF, tag="exm")
                nc.vector.tensor_scalar(exm[:], ex[:], 1.0, None, OP.min)
                k_sb = kvp.tile([P, C], BF, tag="k")
                nc.gpsimd.tensor_tensor(k_sb[:], exm[:], rk[:], OP.add)
                v_sb = v_res[i % VN]
                v_view = v_sb[:].rearrange("p (g c) -> p g c",
                                           c=P + 1)[:, :, 0:P]
                if i % 2 == 0:
                    nc.scalar.activation(
                        v_view, pv[:].rearrange("p (g c) -> p g c", c=P),
                        AF.Copy, scale=IWS)
                else:
                    nc.vector.tensor_scalar_mul(
                        v_view, pv[:].rearrange("p (g c) -> p g c", c=P),
                        IWS)
                for p in range(NPAIR):
                    nc.tensor.matmul(
                        ctx_ps[p][:], k_sb[:, ts(p, P)],
                        v_sb[:, p * (P + 1) : (p + 1) * (P + 1)],
                        start=(i % HT == 0), stop=(i % HT == HT - 1))
                if i == HT - 1:
                    # first-half partial state: overlap its AllReduce with
                    # the second half of phase 1
                    ctx_cp1 = res.tile([P, CP], F32, tag="ctx_cp1")
                    flush_ctx(ctx_cp1, cc_in1, cc_out1)

            ctx_cp2 = res.tile([P, CP], F32, tag="ctx_cp2")
            flush_ctx(ctx_cp2, cc_in2, cc_out2)
            pacc.release()

            def build_state():
                ctxr1 = res.tile([P, CP], F32, tag="ctxr1", name="ctxr1")
                nc.sync.dma_start(
                    ctxr1[:], cc_out1[:].rearrange("(p f) -> p f", p=P))
                ctxr2 = res.tile([P, CP], F32, tag="ctxr2", name="ctxr2")
                nc.sync.dma_start(
                    ctxr2[:], cc_out2[:].rearrange("(p f) -> p f", p=P))
                ctxr = res.tile([P, CP], F32, tag="ctxr", name="ctxr")
                nc.vector.tensor_add(ctxr[:], ctxr1[:], ctxr2[:])
                for p in range(NPAIR):
                    q0 = p * (P + 1)
                    nc.gpsimd.tensor_copy(ctxs_bd[0:D, p * P : p * P + D],
                                          ctxr[0:D, q0 : q0 + D])
                    nc.gpsimd.tensor_copy(
                        ctxs_bd[D:P, p * P + D : (p + 1) * P],
                        ctxr[D:P, q0 + D : q0 + P])
                for kc in range(KC):
                    kq = kc * (P + 1) + P
                    nc.gpsimd.tensor_copy(Ksel[kc][0:D, 2 * kc : 2 * kc + 1],
                                          ctxr[0:D, kq : kq + 1])
                    nc.gpsimd.tensor_copy(
                        Ksel[kc][D:P, 2 * kc + 1 : 2 * kc + 2],
                        ctxr[D:P, kq + 1 - 1 : kq + 1])

            # ---- phase 2b ----
            paux = tc.alloc_tile_pool(name="aux", bufs=1, space="PSUM")

            def stage_a(j):
                den = paux.tile([H, CH], F32, tag="rb", bufs=2, name="den")
                for kc in range(KC):
                    nc.tensor.matmul(den[:], Ksel[kc][:], qte[j][kc][:],
                                     start=(kc == 0), stop=(kc == KC - 1))
                rec = rcp.tile([H, CH], F32, tag="rec")
                nc.vector.reciprocal_approx_fast(rec[:], den[:])
                recb = rcp.tile([H, CH], BF, tag="recb")
                nc.vector.tensor_scalar_mul(recb[:], rec[:], SCALE * XS)
                pxs = []
                for p in range(NPAIR):
                    px = paux.tile([P, CH], F32, tag="px", bufs=3,
                                   name="px")
                    nc.tensor.matmul(px[:], ctxs_bd[:, ts(p, P)],
                                     qte[j][p][:], start=True, stop=True)
                    pxs.append(px)
                xt2 = [xtp.tile([P, 2, CH], F8, tag=f"xt{g}",
                                name=f"xt2_{g}")
                       for g in range(2)]
                for p in range(NPAIR):
                    prb = paux.tile([P, CH], F32, tag="rb", bufs=2,
                                    name="prb")
                    nc.tensor.matmul(prb[:], E_sb[p][:], recb[:],
                                     start=True, stop=True)
                    rb = rcp.tile([P, CH], BF, tag="rbs")
                    if p % 2 == 0:
                        nc.scalar.activation(rb[:], prb[:], AF.Copy)
                    else:
                        nc.vector.tensor_copy(rb[:], prb[:])
                    nc.vector.tensor_mul(xt2[p // 2][:, p % 2, :],
                                         pxs[p][:], rb[:])
                return xt2

            def stage_b(j, xt2):
                o_sb = op_.tile([P, RT, C], BF, tag="o", bufs=3)
                for rt in range(RT):
                    po = pmm.tile([P, C], F32, tag="mm", name="po")
                    for g in range(2):
                        nc.tensor.matmul(po[:], xt2[g][:, :, ts(rt, P)],
                                         pw_sb[g][:], start=(g == 0),
                                         stop=(g == 1), perf_mode=DR)
                    nc.vector.scalar_tensor_tensor(
                        o_sb[:, rt, :], po[:], 1.0 / (WS * XS), bias_sb[:],
                        OP.mult, OP.add)
                nc.sync.dma_start(
                    out_ext[ts(j, CH), :].rearrange("(rt p) c -> p rt c",
                                                    p=P),
                    o_sb[:])

            for j in range(NCH):
                if j == max(NCH - 2, 0):
                    build_state()
                qt_chunk(j)
            pend = []
            for j in range(NCH):
                pend.append((j, stage_a(j)))
                if len(pend) > lookahead:
                    jj, xx = pend.pop(0)
                    stage_b(jj, xx)
            for jj, xx in pend:
                stage_b(jj, xx)
            paux.release()
    nc.compile()
    return nc


def _sub2(a):
    """[C, X] -> [KC2, 128, 2, X] DoubleRow c-subtile layout."""
    X = a.shape[1]
    return np.ascontiguousarray(
        a.reshape(2, 2, 128, X).transpose(0, 2, 1, 3))


def _shard_inputs(target_data, reference_data, q_w, kv_w, proj_w, proj_b,
                  R, ncores):
    bf = ml_dtypes.bfloat16
    f8 = ml_dtypes.float8_e4m3
    kv_wT = np.asarray(kv_w).T.astype(np.float32) * WS
    q_wT = np.asarray(q_w).T.astype(np.float32) * WS
    proj_wT = np.asarray(proj_w).T.astype(np.float32) * WS
    kvw8 = _sub2(kv_wT).astype(f8)
    qw8 = _sub2(q_wT).astype(f8)
    pw8 = _sub2(proj_wT).astype(f8)
    bias_b = np.ascontiguousarray(
        np.broadcast_to(np.asarray(proj_b)[None, :], (128, C))).astype(bf)
    npair = H // 2
    E_const = np.zeros((npair, H, 128), dtype=bf)
    for p in range(npair):
        E_const[p, 2 * p, 0:D] = 1.0
        E_const[p, 2 * p + 1, D:128] = 1.0
    in_maps = []
    for c in range(ncores):
        b, half = divmod(c, 2)
        sl = slice(half * R, (half + 1) * R)
        in_maps.append({
            "refT8": _sub2(np.ascontiguousarray(
                np.asarray(reference_data)[b, sl, :].T)).astype(f8),
            "tgtT8": _sub2(np.ascontiguousarray(
                np.asarray(target_data)[b, sl, :].T)).astype(f8),
            "kvw8": kvw8, "qw8": qw8, "pw8": pw8,
            "bias_b": bias_b, "E_const": E_const,
        })
    return in_maps


def kernel(target_data, reference_data, q_w, kv_w, proj_w, proj_b):
    R = M // 2
    key = (R, NCORES)
    if key not in _CACHE:
        _CACHE[key] = build(R, R, NCORES,
                            [[0, 1], [2, 3], [4, 5], [6, 7]], lookahead=4)
    nc = _CACHE[key]
    in_maps = _shard_inputs(target_data, reference_data, q_w, kv_w, proj_w,
                            proj_b, R, NCORES)
    res = run_bass_kernel_spmd(nc, in_maps, list(range(NCORES)))
    out = np.empty((B, M, C), dtype=np.float32)
    for c in range(NCORES):
        b, half = divmod(c, 2)
        out[b, half * R : (half + 1) * R, :] = np.asarray(
            res.results[c]["out"]).astype(np.float32)
    return out
